# revision 6
# baseline (speedup 1.0000x reference)
"""Bass/Tile MHA kernel for trn2 — builder + host shard/unshard helpers.

Per-core work (8 cores): core c handles batch b=c//2, head-group g=c%2
(8 of 16 heads). Head pairs share 128-partition tiles at bases 0 / 64 so
the K=48 QK^T matmuls land in distinct PE row-groups (free 2x packing),
and the AV matmuls use PSUM column groups 0 / 64 (col packing).

v2 changes vs baseline:
  - FD=1024 "super" tiles for S / exp / P (2-bank PSUM tiles): halves the
    exp-op count and amortizes the per-op engine overhead (ScalarE
    (172+FD)/1.2, DVE (120+FD)/0.96).
  - The three giant qk_sb pad memsets (3 x 27us serial on GpSimd, which
    gated the first attention unit at ~91us) are gone: projection
    evacuations write rows 0:64 per head (psum rows 48-63 are exact zeros
    since the weight columns there are zero), and rows 64-127 are zeroed
    by cheap DVE memsets (or not at all with qk_k64).
  - qk_k64: QK^T matmuls contract only partitions 0:64 (the 48 head dims
    plus 16 zero rows) instead of 128. Same streaming cost, but rows
    64-127 of qk_sb are never read.
  - av is copied PSUM->SBUF right after the last AV matmul so the single
    av PSUM super frees early; normalize runs from SBUF off the PE
    critical path. outT pad rows are written as exact zeros by the
    widened normalize (av pad rows are matmul zeros), no memset.

Dataflow (all matmuls bf16 in / fp32 PSUM accumulate):
  qkT[d_h, t]  = w_qk^T x           (lhsT=w_qk tile, rhs=x^T tile)
  V[t, d_v]    = x w_v              (lhsT=x^T tile, rhs=w_v)
  S^T[k, q]    = (K^T)^T Q^T        (K=48 contraction, row-packed pairs)
  P^T          = exp(S^T)           (ScalarE true exp / VectorE fast-exp)
  outT'[d,q],l = (V|1)^T P^T        (ones column gives softmax denoms)
  outT         = outT' * bcast(1/l) + b_v
  y[t, j]      = outT^T w_out       (+ b_out and cross-core sum on host)
"""

import math

import numpy as np
import ml_dtypes

import concourse.bass as bass
import concourse.mybir as mybir
import concourse.tile as tile
from concourse import bacc

F32 = mybir.dt.float32
BF16 = mybir.dt.bfloat16
I16 = mybir.dt.int16
AF = mybir.ActivationFunctionType
OP = mybir.AluOpType

DIM = 768
PH = 48
NP = 4          # head pairs per core
HC = 8          # heads per core
NDT = DIM // 128  # 6 contraction tiles for the projections

# Schraudolph fast-exp in bf16 bit space: bits = round(x*128/ln2 + (127*128 - C))
SCH_A = 128.0 / math.log(2.0)
SCH_C = 4.7
# +0.5: the fp32->int16 convert truncates, this re-centers it to round-nearest
SCH_B = 127.0 * 128.0 - SCH_C + 0.5


def _spread(n, total=32):
    """n slot indices spread evenly over range(total) (Bresenham)."""
    return frozenset(s for s in range(total)
                     if (s + 1) * n // total > s * n // total)


def build_kernel(T=2048, num_devices=8, qk_k64=True, qk_pair=False,
                 dve_n_era0=16, dve_n_era1=13):
    """Returns compiled Bacc module.

    qk_k64: QK^T matmuls use a 64-partition contraction AP (rows 48-63
    zeroed by the evacs; rows 64-127 never read -> no memset). False falls
    back to 128-contraction with DVE memsets for rows 64-127.
    qk_pair: store Q^T/K^T pair-interleaved (head A rows 0-47, head B
    64-111, same layout as the projection psum): ONE 128-row evac per
    (pair, qk, sup), and the two heads' QK^T matmuls land in different PE
    row-groups (tile_position (0,*) / (64,*)) so they run concurrently.
    Implies qk_k64-style 64-row contraction APs.
    dve_n_era*: of the 32 exp slots per attention unit, how many run as
    VectorE fast-exp (era0 = units with interleaved projections, era1 =
    the rest); the remainder run ScalarE true exp.
    """
    KT = T // 128                 # k-tiles (token tiles)
    QCW = 1024                    # q super-chunk (2 PSUM banks for S)
    NQG = T // QCW                # q groups per head pair
    KP = 64 if qk_k64 else 128    # QK contraction partitions
    LEAD = 2                      # AV trails QK by 2 k-tiles

    dve_sets = (_spread(dve_n_era0), _spread(dve_n_era1))

    nc = bacc.Bacc("TRN2", target_bir_lowering=False, debug=False,
                   num_devices=num_devices)

    xt_d = nc.dram_tensor("xt", (DIM, T), BF16, kind="ExternalInput")
    wqk_d = nc.dram_tensor("wqk", (DIM, NP * 2 * 128), BF16, kind="ExternalInput")
    wv_d = nc.dram_tensor("wv", (DIM, HC * PH), BF16, kind="ExternalInput")
    wo_d = nc.dram_tensor("wo", (NP * 128, DIM), BF16, kind="ExternalInput")
    bqk_d = nc.dram_tensor("bqk", (128, NP * 2), F32, kind="ExternalInput")
    y_d = nc.dram_tensor("y", (T, DIM), F32, kind="ExternalOutput")

    with tile.TileContext(nc) as tc:
        with (
            tc.tile_pool(name="const", bufs=1) as cpool,
            tc.tile_pool(name="pt", bufs=8) as ptpool,
            tc.tile_pool(name="avs", bufs=4) as aspool,
            tc.tile_pool(name="norm", bufs=2) as npool,
            tc.tile_pool(name="ysb", bufs=2) as ypool,
            tc.tile_pool(name="st", bufs=3, space="PSUM") as stpool,
            tc.tile_pool(name="av", bufs=1, space="PSUM") as avpool,
        ):
            # ---- persistent SBUF tensors ----
            xt_sb = cpool.tile([128, NDT, T], BF16, tag="xt")
            wqk_sb = cpool.tile([128, NDT, NP * 2 * 128], BF16, tag="wqk")
            wv_sb = cpool.tile([128, NDT, HC * PH], BF16, tag="wv")
            wo_sb = cpool.tile([128, NP, DIM], BF16, tag="wo")
            bqk_sb = cpool.tile([128, NP * 2], F32, tag="bqk")
            # per-head Q^T/K^T; rows 0-47 data, rows 48-63 zeroed by the
            # projection evacs. Rows 64-127 only matter when KP=128.
            # qk_pair: [128, NP, 2, T] pair-interleaved instead (A rows
            # 0-47, B rows 64-111, pads zero) straight from the psum layout.
            if qk_pair:
                qk_sb = cpool.tile([128, NP, 2, T], BF16, tag="qk")
            else:
                qk_sb = cpool.tile([128, HC, 2, T], BF16, tag="qk")
            # V' columns per head: 0 = ones (softmax denominator lands on
            # PSUM row 0 / 64 of the shared bank), 1-48 = V, 49-63 = zero
            v_sb = cpool.tile([128, KT, HC, 64], BF16, tag="v")
            outT_sb = cpool.tile([128, NP, T], BF16, tag="outT")

            # ---- input DMAs ----
            for dt_i in range(NDT):
                nc.sync.dma_start(xt_sb[:, dt_i, :], xt_d[dt_i * 128:(dt_i + 1) * 128, :])
                nc.sync.dma_start(wqk_sb[:, dt_i, :], wqk_d[dt_i * 128:(dt_i + 1) * 128, :])
                nc.sync.dma_start(wv_sb[:, dt_i, :], wv_d[dt_i * 128:(dt_i + 1) * 128, :])
            for p in range(NP):
                nc.sync.dma_start(wo_sb[:, p, :], wo_d[p * 128:(p + 1) * 128, :])
            nc.sync.dma_start(bqk_sb[:], bqk_d[:])

            # ones column for the softmax-denominator trick; zero pads
            nc.gpsimd.memset(v_sb[:, :, :, 0:1], 1.0)
            nc.gpsimd.memset(v_sb[:, :, :, PH + 1:64], 0.0)
            if not qk_k64 and not qk_pair:
                # rows 64-127 feed the padded K=128 contraction: zero them
                # on the (otherwise idle) DVE in four chunks
                for h4 in range(4):
                    nc.vector.memset(qk_sb[64:128, h4 * 2:(h4 + 1) * 2, :, :], 0.0)

            def qkT_proj(p):
                # qkT[d_h, t] for pair p: psum rows 0-47 head A dims (48-63
                # zero weight cols), 64-111 head B (112-127 zero)
                for qk in range(2):
                    col0 = (p * 2 + qk) * 128
                    col = p * 2 + qk
                    for sup in range(T // QCW):
                        ps = stpool.tile([128, QCW], F32, tag="st", name="pj")
                        for qc in range(2):
                            for dt_i in range(NDT):
                                for mh in range(2):
                                    nc.tensor.matmul(
                                        ps[mh * 64:(mh + 1) * 64,
                                           qc * 512:(qc + 1) * 512],
                                        wqk_sb[:, dt_i, col0 + mh * 64:col0 + (mh + 1) * 64],
                                        xt_sb[:, dt_i,
                                              sup * QCW + qc * 512:sup * QCW + (qc + 1) * 512],
                                        start=(dt_i == 0), stop=(dt_i == NDT - 1),
                                        skip_group_check=True)
                        tsl = np.s_[sup * QCW:(sup + 1) * QCW]
                        if qk_pair:
                            # single 128-row evac: psum layout IS the
                            # pair-interleaved layout, pads exact zeros
                            nc.scalar.activation(
                                qk_sb[:, p, qk, tsl], ps[:],
                                AF.Identity, bias=bqk_sb[:, col:col + 1])
                        else:
                            # widened evacs: rows 48-63 of each head slot
                            # get exact zeros (zero w cols + zero bias rows)
                            nc.scalar.activation(
                                qk_sb[0:64, p * 2, qk, tsl], ps[0:64, :],
                                AF.Identity, bias=bqk_sb[0:64, col:col + 1])
                            nc.scalar.activation(
                                qk_sb[0:64, p * 2 + 1, qk, tsl], ps[64:128, :],
                                AF.Identity, bias=bqk_sb[64:128, col:col + 1])

            def v_proj():
                for ts2 in range(KT // 2):
                    psb = stpool.tile([128, QCW], F32, tag="st", name="vp")
                    for sub in range(2):
                        tt = ts2 * 2 + sub
                        for dt_i in range(NDT):
                            for mh in range(2):
                                nc.tensor.matmul(
                                    psb[mh * 64:(mh + 1) * 64,
                                        sub * 512:sub * 512 + HC * PH],
                                    xt_sb[:, dt_i, tt * 128 + mh * 64:tt * 128 + (mh + 1) * 64],
                                    wv_sb[:, dt_i, :],
                                    start=(dt_i == 0), stop=(dt_i == NDT - 1),
                                    skip_group_check=True)
                    src = (psb[:].rearrange("p (s c) -> p s c", s=2)
                           [:, :, 0:HC * PH]
                           .rearrange("p s (h d) -> p s h d", h=HC))
                    nc.scalar.activation(
                        v_sb[:, ts2 * 2:ts2 * 2 + 2, :, 1:PH + 1], src, AF.Copy)

            def attention(p, qg, era):
                cs = np.s_[qg * QCW:(qg + 1) * QCW]
                dve_set = dve_sets[era]
                av = avpool.tile([128, QCW], F32, tag="av", name="av")
                pts = {}

                def qk_emit(kt):
                    for hh in range(2):
                        st = stpool.tile([128, QCW], F32, tag="st")
                        for qc in range(2):
                            for mh in range(2):
                                if qk_pair:
                                    b0 = hh * 64
                                    lhsT = qk_sb[b0:b0 + 64, p, 1,
                                                 kt * 128 + mh * 64:kt * 128 + (mh + 1) * 64]
                                    rhs = qk_sb[b0:b0 + 64, p, 0,
                                                qg * QCW + qc * 512:qg * QCW + (qc + 1) * 512]
                                else:
                                    lhsT = qk_sb[0:KP, p * 2 + hh, 1,
                                                 kt * 128 + mh * 64:kt * 128 + (mh + 1) * 64]
                                    rhs = qk_sb[0:KP, p * 2 + hh, 0,
                                                qg * QCW + qc * 512:qg * QCW + (qc + 1) * 512]
                                nc.tensor.matmul(
                                    st[mh * 64:(mh + 1) * 64,
                                       qc * 512:(qc + 1) * 512],
                                    lhsT, rhs,
                                    start=True, stop=True,
                                    skip_group_check=True)
                        pt = ptpool.tile([128, QCW], BF16, tag="pt")
                        if (kt * 2 + hh) % 32 in dve_set:
                            nc.vector.tensor_scalar(
                                pt[:].bitcast(I16), st[:], SCH_A, SCH_B,
                                OP.mult, OP.add)
                        else:
                            nc.scalar.activation(pt[:], st[:], AF.Exp)
                        pts[(kt, hh)] = pt

                def av_emit(kt):
                    for hh in range(2):
                        pt = pts.pop((kt, hh))
                        for qc in range(2):
                            nc.tensor.matmul(
                                av[hh * 64:(hh + 1) * 64, qc * 512:(qc + 1) * 512],
                                v_sb[:, kt, p * 2 + hh, :],
                                pt[:, qc * 512:(qc + 1) * 512],
                                start=(kt == 0), stop=(kt == KT - 1),
                                skip_group_check=True)

                for kt in range(KT + LEAD):
                    if kt < KT:
                        qk_emit(kt)
                    if kt >= LEAD:
                        av_emit(kt - LEAD)

                # stage both av halves to base-0 SBUF tiles right away: the
                # single av PSUM super frees for the next unit, and every
                # later SB+SB tensor_tensor has base-aligned inputs (walrus
                # requires equal base partitions when both inputs are SBUF)
                avsa = aspool.tile([128, QCW], F32, tag="avs", name="avsa")
                avsb = aspool.tile([128, QCW], F32, tag="avs", name="avsb")
                if p % 2 == 0:
                    nc.scalar.activation(avsa[0:64, :], av[0:64, :], AF.Copy)
                    nc.vector.tensor_copy(avsb[0:64, :], av[64:128, :])
                else:
                    nc.vector.tensor_copy(avsa[0:64, :], av[0:64, :])
                    nc.scalar.activation(avsb[0:64, :], av[64:128, :], AF.Copy)

                # normalize + bias into outT (denominators live in row 0)
                r2a = npool.tile([128, QCW], F32, tag="r2", name="r2a")
                r2b = npool.tile([128, QCW], F32, tag="r2", name="r2b")
                rbca = npool.tile([128, QCW], F32, tag="rbc", name="rbca")
                rbcb = npool.tile([128, QCW], F32, tag="rbc", name="rbcb")
                nc.vector.reciprocal_approx_fast(r2a[0:1, :], avsa[0:1, :])
                nc.vector.reciprocal_approx_fast(r2b[0:1, :], avsb[0:1, :])
                # widened to 64 rows: avs pad rows are exact matmul zeros,
                # so outT pad rows become exact zeros (no memset needed)
                nc.gpsimd.partition_broadcast(rbca[0:64, :], r2a[0:1, :])
                nc.gpsimd.partition_broadcast(rbcb[0:64, :], r2b[0:1, :])
                nc.vector.tensor_mul(outT_sb[0:64, p, cs],
                                     avsa[0:64, :], rbca[0:64, :])
                nc.vector.tensor_mul(outT_sb[64:128, p, cs],
                                     avsb[0:64, :], rbcb[0:64, :])

            def final_proj(qg):
                for tt in range(QCW // 128):
                    t0 = qg * QCW + tt * 128
                    ysb = ypool.tile([128, DIM], F32, tag="ysb")
                    psb = stpool.tile([128, QCW], F32, tag="st", name="yp")
                    for jc in range(2):
                        for pp in range(NP):
                            for mh in range(2):
                                nc.tensor.matmul(
                                    psb[mh * 64:(mh + 1) * 64,
                                        jc * 512:jc * 512 + 384],
                                    outT_sb[:, pp, t0 + mh * 64:t0 + (mh + 1) * 64],
                                    wo_sb[:, pp, jc * 384:(jc + 1) * 384],
                                    start=(pp == 0), stop=(pp == NP - 1),
                                    skip_group_check=True)
                    src = psb[:].rearrange("p (j c) -> p j c", j=2)[:, :, 0:384]
                    dst = ysb[:].rearrange("p (j c) -> p j c", j=2)
                    if tt % 2 == 0:
                        nc.scalar.activation(dst, src, AF.Copy)
                    else:
                        nc.vector.tensor_copy(dst, src)
                    nc.sync.dma_start(y_d[t0:t0 + 128, :], ysb[:])

            # ---- emission order (scheduling priority) ----
            qkT_proj(0)
            v_proj()
            for qg in range(NQG):
                for p in range(NP):
                    if qg == 0 and p + 1 < NP:
                        qkT_proj(p + 1)
                    attention(p, qg, era=0 if qg == 0 else 1)
                final_proj(qg)

    nc.compile()
    return nc


# ---------------- host-side sharding ----------------

def host_prep(x, w_in, b_in, w_out, T=2048):
    """Full inputs -> list of 8 per-core input dicts."""
    scale = 1.0 / math.sqrt(PH)
    wr = np.asarray(w_in).reshape(DIM, 16, 3, PH)
    br = np.asarray(b_in).reshape(16, 3, PH)
    wog = np.asarray(w_out)  # (768, 768), row dv = h*48+d
    in_maps = []
    for c in range(8):
        b, g = divmod(c, 2)
        wqk = np.zeros((DIM, NP * 2 * 128), np.float32)
        bqk = np.zeros((128, NP * 2), np.float32)
        wv = np.zeros((DIM, HC * PH), np.float32)
        wo = np.zeros((NP * 128, DIM), np.float32)
        for p in range(NP):
            for hh, base in ((0, 0), (1, 64)):
                gh = g * 8 + p * 2 + hh
                wqk[:, (p * 2) * 128 + base:(p * 2) * 128 + base + PH] = wr[:, gh, 0] * scale
                wqk[:, (p * 2 + 1) * 128 + base:(p * 2 + 1) * 128 + base + PH] = wr[:, gh, 1]
                bqk[base:base + PH, p * 2] = br[gh, 0] * scale
                bqk[base:base + PH, p * 2 + 1] = br[gh, 1]
                wv[:, (p * 2 + hh) * PH:(p * 2 + hh + 1) * PH] = wr[:, gh, 2]
                wo[p * 128 + base + 1:p * 128 + base + 1 + PH, :] = wog[gh * PH:(gh + 1) * PH, :]
        in_maps.append({
            "xt": np.ascontiguousarray(np.asarray(x)[b].T).astype(ml_dtypes.bfloat16),
            "wqk": wqk.astype(ml_dtypes.bfloat16),
            "wv": wv.astype(ml_dtypes.bfloat16),
            "wo": wo.astype(ml_dtypes.bfloat16),
            "bqk": bqk,
        })
    return in_maps


def host_post(results, b_out, b_in, w_out, B=4, T=2048):
    # the V bias contributes bv @ w_out, a per-column constant: add on host
    bv_all = np.asarray(b_in).reshape(16, 3, PH)[:, 2, :].reshape(DIM)
    const = np.asarray(b_out) + bv_all @ np.asarray(w_out)
    out = np.empty((B, T, DIM), np.float32)
    for b in range(B):
        out[b] = results[2 * b]["y"] + results[2 * b + 1]["y"] + const[None, :]
    return out


# ---------------- self-contained kernel() entry point ----------------

_CACHED = {}


def _get_nc():
    if "nc" not in _CACHED:
        _CACHED["nc"] = build_kernel(T=2048, num_devices=8)
    return _CACHED["nc"]


def kernel(x, w_in, b_in, w_out, b_out):
    """Full-input MHA forward on 8 NeuronCores.

    x: (4, 2048, 768) f32; w_in: (768, 2304); b_in: (2304,);
    w_out: (768, 768); b_out: (768,). Returns (4, 2048, 768) f32.
    """
    from concourse.bass_utils import run_bass_kernel_spmd

    x = np.asarray(x, np.float32)
    w_in = np.asarray(w_in, np.float32)
    b_in = np.asarray(b_in, np.float32)
    w_out = np.asarray(w_out, np.float32)
    b_out = np.asarray(b_out, np.float32)

    nc = _get_nc()
    in_maps = host_prep(x, w_in, b_in, w_out, T=2048)
    res = run_bass_kernel_spmd(nc, in_maps, core_ids=list(range(8)))
    return host_post(res.results, b_out, b_in, w_out, B=4, T=2048)


# revision 7
# speedup vs baseline: 1.3967x; 1.3967x over previous
"""Bass/Tile MHA kernel for trn2 — builder + host shard/unshard helpers.

Per-core work (8 cores): core c handles batch b=c//2, head-group g=c%2
(8 of 16 heads). Head pairs share 128-partition tiles at bases 0 / 64 so
the K=48 QK^T matmuls land in distinct PE row-groups (free 2x packing),
and the AV matmuls use PSUM column groups 0 / 64 (col packing).

v2 changes vs baseline:
  - FD=1024 "super" tiles for S / exp / P (2-bank PSUM tiles): halves the
    exp-op count and amortizes the per-op engine overhead (ScalarE
    (172+FD)/1.2, DVE (120+FD)/0.96).
  - The three giant qk_sb pad memsets (3 x 27us serial on GpSimd, which
    gated the first attention unit at ~91us) are gone: projection
    evacuations write rows 0:64 per head (psum rows 48-63 are exact zeros
    since the weight columns there are zero), and rows 64-127 are zeroed
    by cheap DVE memsets (or not at all with qk_k64).
  - qk_k64: QK^T matmuls contract only partitions 0:64 (the 48 head dims
    plus 16 zero rows) instead of 128. Same streaming cost, but rows
    64-127 of qk_sb are never read.
  - av is copied PSUM->SBUF right after the last AV matmul so the single
    av PSUM super frees early; normalize runs from SBUF off the PE
    critical path. outT pad rows are written as exact zeros by the
    widened normalize (av pad rows are matmul zeros), no memset.

Dataflow (all matmuls bf16 in / fp32 PSUM accumulate):
  qkT[d_h, t]  = w_qk^T x           (lhsT=w_qk tile, rhs=x^T tile)
  V[t, d_v]    = x w_v              (lhsT=x^T tile, rhs=w_v)
  S^T[k, q]    = (K^T)^T Q^T        (K=48 contraction, row-packed pairs)
  P^T          = exp(S^T)           (ScalarE true exp / VectorE fast-exp)
  outT'[d,q],l = (V|1)^T P^T        (ones column gives softmax denoms)
  outT         = outT' * bcast(1/l) + b_v
  y[t, j]      = outT^T w_out       (+ b_out and cross-core sum on host)
"""

import math

import numpy as np
import ml_dtypes

import concourse.bass as bass
import concourse.mybir as mybir
import concourse.tile as tile
from concourse import bacc

F32 = mybir.dt.float32
BF16 = mybir.dt.bfloat16
I16 = mybir.dt.int16
AF = mybir.ActivationFunctionType
OP = mybir.AluOpType

DIM = 768
PH = 48
NP = 4          # head pairs per core
HC = 8          # heads per core
NDT = DIM // 128  # 6 contraction tiles for the projections

# Schraudolph fast-exp in bf16 bit space: bits = round(x*128/ln2 + (127*128 - C))
SCH_A = 128.0 / math.log(2.0)
SCH_C = 4.7
# +0.5: the fp32->int16 convert truncates, this re-centers it to round-nearest
SCH_B = 127.0 * 128.0 - SCH_C + 0.5


def _spread(n, total=32):
    """n slot indices spread evenly over range(total) (Bresenham)."""
    return frozenset(s for s in range(total)
                     if (s + 1) * n // total > s * n // total)


def build_kernel(T=2048, num_devices=8, qk_k64=True, qk_pair=False,
                 dve_n_era0=16, dve_n_era1=13):
    """Returns compiled Bacc module.

    qk_k64: QK^T matmuls use a 64-partition contraction AP (rows 48-63
    zeroed by the evacs; rows 64-127 never read -> no memset). False falls
    back to 128-contraction with DVE memsets for rows 64-127.
    qk_pair: store Q^T/K^T pair-interleaved (head A rows 0-47, head B
    64-111, same layout as the projection psum): ONE 128-row evac per
    (pair, qk, sup), and the two heads' QK^T matmuls land in different PE
    row-groups (tile_position (0,*) / (64,*)) so they run concurrently.
    Implies qk_k64-style 64-row contraction APs.
    dve_n_era*: of the 32 exp slots per attention unit, how many run as
    VectorE fast-exp (era0 = units with interleaved projections, era1 =
    the rest); the remainder run ScalarE true exp.
    """
    KT = T // 128                 # k-tiles (token tiles)
    QCW = 1024                    # q super-chunk (2 PSUM banks for S)
    NQG = T // QCW                # q groups per head pair
    KP = 64 if qk_k64 else 128    # QK contraction partitions
    LEAD = 2                      # AV trails QK by 2 k-tiles

    dve_sets = (_spread(dve_n_era0), _spread(dve_n_era1))

    nc = bacc.Bacc("TRN2", target_bir_lowering=False, debug=False,
                   num_devices=num_devices)

    xt_d = nc.dram_tensor("xt", (DIM, T), BF16, kind="ExternalInput")
    wqk_d = nc.dram_tensor("wqk", (DIM, NP * 2 * 128), BF16, kind="ExternalInput")
    wv_d = nc.dram_tensor("wv", (DIM, HC * PH), BF16, kind="ExternalInput")
    wo_d = nc.dram_tensor("wo", (NP * 128, DIM), BF16, kind="ExternalInput")
    bqk_d = nc.dram_tensor("bqk", (128, NP * 2), F32, kind="ExternalInput")
    y_d = nc.dram_tensor("y", (T, DIM), F32, kind="ExternalOutput")

    with tile.TileContext(nc) as tc:
        with (
            tc.tile_pool(name="const", bufs=1) as cpool,
            tc.tile_pool(name="pt", bufs=8) as ptpool,
            tc.tile_pool(name="avs", bufs=4) as aspool,
            tc.tile_pool(name="norm", bufs=2) as npool,
            tc.tile_pool(name="ysb", bufs=2) as ypool,
            tc.tile_pool(name="st", bufs=3, space="PSUM") as stpool,
            tc.tile_pool(name="av", bufs=1, space="PSUM") as avpool,
        ):
            # ---- persistent SBUF tensors ----
            xt_sb = cpool.tile([128, NDT, T], BF16, tag="xt")
            wqk_sb = cpool.tile([128, NDT, NP * 2 * 128], BF16, tag="wqk")
            wv_sb = cpool.tile([128, NDT, HC * PH], BF16, tag="wv")
            wo_sb = cpool.tile([128, NP, DIM], BF16, tag="wo")
            bqk_sb = cpool.tile([128, NP * 2], F32, tag="bqk")
            # per-head Q^T/K^T; rows 0-47 data, rows 48-63 zeroed by the
            # projection evacs. Rows 64-127 only matter when KP=128.
            # qk_pair: [128, NP, 2, T] pair-interleaved instead (A rows
            # 0-47, B rows 64-111, pads zero) straight from the psum layout.
            if qk_pair:
                qk_sb = cpool.tile([128, NP, 2, T], BF16, tag="qk")
            else:
                qk_sb = cpool.tile([128, HC, 2, T], BF16, tag="qk")
            # V' columns per head: 0 = ones (softmax denominator lands on
            # PSUM row 0 / 64 of the shared bank), 1-48 = V, 49-63 = zero
            v_sb = cpool.tile([128, KT, HC, 64], BF16, tag="v")
            outT_sb = cpool.tile([128, NP, T], BF16, tag="outT")

            # ---- input DMAs ----
            for dt_i in range(NDT):
                nc.sync.dma_start(wqk_sb[:, dt_i, 0:256],
                                  wqk_d[dt_i * 128:(dt_i + 1) * 128, 0:256])
                nc.sync.dma_start(xt_sb[:, dt_i, 0:QCW],
                                  xt_d[dt_i * 128:(dt_i + 1) * 128, 0:QCW])
            for dt_i in range(NDT):
                nc.sync.dma_start(xt_sb[:, dt_i, QCW:T],
                                  xt_d[dt_i * 128:(dt_i + 1) * 128, QCW:T])
                nc.sync.dma_start(wqk_sb[:, dt_i, 256:],
                                  wqk_d[dt_i * 128:(dt_i + 1) * 128, 256:])
                nc.sync.dma_start(wv_sb[:, dt_i, :], wv_d[dt_i * 128:(dt_i + 1) * 128, :])
            for p in range(NP):
                nc.sync.dma_start(wo_sb[:, p, :], wo_d[p * 128:(p + 1) * 128, :])
            nc.sync.dma_start(bqk_sb[:], bqk_d[:])

            # ones column for the softmax-denominator trick; zero pads
            nc.gpsimd.memset(v_sb[:, :, :, 0:1], 1.0)
            nc.gpsimd.memset(v_sb[:, :, :, PH + 1:64], 0.0)
            if not qk_k64 and not qk_pair:
                # rows 64-127 feed the padded K=128 contraction: zero them
                # on the (otherwise idle) DVE in four chunks
                for h4 in range(4):
                    nc.vector.memset(qk_sb[64:128, h4 * 2:(h4 + 1) * 2, :, :], 0.0)

            def qkT_proj(p):
                # qkT[d_h, t] for pair p: psum rows 0-47 head A dims (48-63
                # zero weight cols), 64-111 head B (112-127 zero)
                for qk in range(2):
                    col0 = (p * 2 + qk) * 128
                    col = p * 2 + qk
                    for sup in range(T // QCW):
                        ps = stpool.tile([128, QCW], F32, tag="st", name="pj")
                        for qc in range(2):
                            for dt_i in range(NDT):
                                nc.tensor.matmul(
                                    ps[:, qc * 512:(qc + 1) * 512],
                                    wqk_sb[:, dt_i, col0:col0 + 128],
                                    xt_sb[:, dt_i,
                                          sup * QCW + qc * 512:sup * QCW + (qc + 1) * 512],
                                    start=(dt_i == 0), stop=(dt_i == NDT - 1),
                                    skip_group_check=True)
                        tsl = np.s_[sup * QCW:(sup + 1) * QCW]
                        if qk_pair:
                            # single 128-row evac: psum layout IS the
                            # pair-interleaved layout, pads exact zeros
                            nc.scalar.activation(
                                qk_sb[:, p, qk, tsl], ps[:],
                                AF.Identity, bias=bqk_sb[:, col:col + 1])
                        else:
                            # widened evacs: rows 48-63 of each head slot
                            # get exact zeros (zero w cols + zero bias rows)
                            nc.scalar.activation(
                                qk_sb[0:64, p * 2, qk, tsl], ps[0:64, :],
                                AF.Identity, bias=bqk_sb[0:64, col:col + 1])
                            nc.scalar.activation(
                                qk_sb[0:64, p * 2 + 1, qk, tsl], ps[64:128, :],
                                AF.Identity, bias=bqk_sb[64:128, col:col + 1])

            def v_proj():
                for ts2 in range(KT // 2):
                    psb = stpool.tile([128, QCW], F32, tag="st", name="vp")
                    for sub in range(2):
                        tt = ts2 * 2 + sub
                        for dt_i in range(NDT):
                            nc.tensor.matmul(
                                psb[:, sub * 512:sub * 512 + HC * PH],
                                xt_sb[:, dt_i, tt * 128:(tt + 1) * 128],
                                wv_sb[:, dt_i, :],
                                start=(dt_i == 0), stop=(dt_i == NDT - 1),
                                skip_group_check=True)
                    src = (psb[:].rearrange("p (s c) -> p s c", s=2)
                           [:, :, 0:HC * PH]
                           .rearrange("p s (h d) -> p s h d", h=HC))
                    nc.scalar.activation(
                        v_sb[:, ts2 * 2:ts2 * 2 + 2, :, 1:PH + 1], src, AF.Copy)

            def attention(p, qg, era):
                cs = np.s_[qg * QCW:(qg + 1) * QCW]
                dve_set = dve_sets[era]
                av = avpool.tile([128, QCW], F32, tag="av", name="av")
                pts = {}

                def qk_emit(kt):
                    for hh in range(2):
                        st = stpool.tile([128, QCW], F32, tag="st")
                        for qc in range(2):
                            if qk_pair:
                                b0 = hh * 64
                                lhsT = qk_sb[b0:b0 + 64, p, 1,
                                             kt * 128:(kt + 1) * 128]
                                rhs = qk_sb[b0:b0 + 64, p, 0,
                                            qg * QCW + qc * 512:qg * QCW + (qc + 1) * 512]
                            else:
                                lhsT = qk_sb[0:KP, p * 2 + hh, 1,
                                             kt * 128:(kt + 1) * 128]
                                rhs = qk_sb[0:KP, p * 2 + hh, 0,
                                            qg * QCW + qc * 512:qg * QCW + (qc + 1) * 512]
                            nc.tensor.matmul(
                                st[:, qc * 512:(qc + 1) * 512],
                                lhsT, rhs,
                                start=True, stop=True,
                                skip_group_check=True)
                        pt = ptpool.tile([128, QCW], BF16, tag="pt")
                        if (kt * 2 + hh) % 32 in dve_set:
                            nc.vector.tensor_scalar(
                                pt[:].bitcast(I16), st[:], SCH_A, SCH_B,
                                OP.mult, OP.add)
                        else:
                            nc.scalar.activation(pt[:], st[:], AF.Exp)
                        pts[(kt, hh)] = pt

                def av_emit(kt):
                    for hh in range(2):
                        pt = pts.pop((kt, hh))
                        for qc in range(2):
                            nc.tensor.matmul(
                                av[hh * 64:(hh + 1) * 64, qc * 512:(qc + 1) * 512],
                                v_sb[:, kt, p * 2 + hh, :],
                                pt[:, qc * 512:(qc + 1) * 512],
                                start=(kt == 0), stop=(kt == KT - 1),
                                skip_group_check=True)

                for kt in range(KT + LEAD):
                    if kt < KT:
                        qk_emit(kt)
                    if kt >= LEAD:
                        av_emit(kt - LEAD)

                # stage both av halves to base-0 SBUF tiles right away: the
                # single av PSUM super frees for the next unit, and every
                # later SB+SB tensor_tensor has base-aligned inputs (walrus
                # requires equal base partitions when both inputs are SBUF)
                avsa = aspool.tile([128, QCW], F32, tag="avs", name="avsa")
                avsb = aspool.tile([128, QCW], F32, tag="avs", name="avsb")
                if p % 2 == 0:
                    nc.scalar.activation(avsa[0:64, :], av[0:64, :], AF.Copy)
                    nc.vector.tensor_copy(avsb[0:64, :], av[64:128, :])
                else:
                    nc.vector.tensor_copy(avsa[0:64, :], av[0:64, :])
                    nc.scalar.activation(avsb[0:64, :], av[64:128, :], AF.Copy)

                # normalize + bias into outT (denominators live in row 0)
                r2a = npool.tile([128, QCW], F32, tag="r2", name="r2a")
                r2b = npool.tile([128, QCW], F32, tag="r2", name="r2b")
                rbca = npool.tile([128, QCW], F32, tag="rbc", name="rbca")
                rbcb = npool.tile([128, QCW], F32, tag="rbc", name="rbcb")
                nc.vector.reciprocal_approx_fast(r2a[0:1, :], avsa[0:1, :])
                nc.vector.reciprocal_approx_fast(r2b[0:1, :], avsb[0:1, :])
                # widened to 64 rows: avs pad rows are exact matmul zeros,
                # so outT pad rows become exact zeros (no memset needed)
                nc.gpsimd.partition_broadcast(rbca[0:64, :], r2a[0:1, :])
                nc.gpsimd.partition_broadcast(rbcb[0:64, :], r2b[0:1, :])
                nc.vector.tensor_mul(outT_sb[0:64, p, cs],
                                     avsa[0:64, :], rbca[0:64, :])
                nc.vector.tensor_mul(outT_sb[64:128, p, cs],
                                     avsb[0:64, :], rbcb[0:64, :])

            def final_proj(qg):
                for tt in range(QCW // 128):
                    t0 = qg * QCW + tt * 128
                    ysb = ypool.tile([128, DIM], F32, tag="ysb")
                    psb = stpool.tile([128, QCW], F32, tag="st", name="yp")
                    for jc in range(2):
                        for pp in range(NP):
                            nc.tensor.matmul(
                                psb[:, jc * 512:jc * 512 + 384],
                                outT_sb[:, pp, t0:t0 + 128],
                                wo_sb[:, pp, jc * 384:(jc + 1) * 384],
                                start=(pp == 0), stop=(pp == NP - 1),
                                skip_group_check=True)
                    src = psb[:].rearrange("p (j c) -> p j c", j=2)[:, :, 0:384]
                    dst = ysb[:].rearrange("p (j c) -> p j c", j=2)
                    if tt % 2 == 0:
                        nc.scalar.activation(dst, src, AF.Copy)
                    else:
                        nc.vector.tensor_copy(dst, src)
                    nc.sync.dma_start(y_d[t0:t0 + 128, :], ysb[:])

            # ---- emission order (scheduling priority) ----
            qkT_proj(0)
            v_proj()
            for qg in range(NQG):
                for p in range(NP):
                    if qg == 0 and p + 1 < NP:
                        qkT_proj(p + 1)
                    attention(p, qg, era=0 if qg == 0 else 1)
                final_proj(qg)

    nc.compile()
    return nc


# ---------------- host-side sharding ----------------

def host_prep(x, w_in, b_in, w_out, T=2048):
    """Full inputs -> list of 8 per-core input dicts."""
    scale = 1.0 / math.sqrt(PH)
    wr = np.asarray(w_in).reshape(DIM, 16, 3, PH)
    br = np.asarray(b_in).reshape(16, 3, PH)
    wog = np.asarray(w_out)  # (768, 768), row dv = h*48+d
    in_maps = []
    for c in range(8):
        b, g = divmod(c, 2)
        wqk = np.zeros((DIM, NP * 2 * 128), np.float32)
        bqk = np.zeros((128, NP * 2), np.float32)
        wv = np.zeros((DIM, HC * PH), np.float32)
        wo = np.zeros((NP * 128, DIM), np.float32)
        for p in range(NP):
            for hh, base in ((0, 0), (1, 64)):
                gh = g * 8 + p * 2 + hh
                wqk[:, (p * 2) * 128 + base:(p * 2) * 128 + base + PH] = wr[:, gh, 0] * scale
                wqk[:, (p * 2 + 1) * 128 + base:(p * 2 + 1) * 128 + base + PH] = wr[:, gh, 1]
                bqk[base:base + PH, p * 2] = br[gh, 0] * scale
                bqk[base:base + PH, p * 2 + 1] = br[gh, 1]
                wv[:, (p * 2 + hh) * PH:(p * 2 + hh + 1) * PH] = wr[:, gh, 2]
                wo[p * 128 + base + 1:p * 128 + base + 1 + PH, :] = wog[gh * PH:(gh + 1) * PH, :]
        in_maps.append({
            "xt": np.ascontiguousarray(np.asarray(x)[b].T).astype(ml_dtypes.bfloat16),
            "wqk": wqk.astype(ml_dtypes.bfloat16),
            "wv": wv.astype(ml_dtypes.bfloat16),
            "wo": wo.astype(ml_dtypes.bfloat16),
            "bqk": bqk,
        })
    return in_maps


def host_post(results, b_out, b_in, w_out, B=4, T=2048):
    # the V bias contributes bv @ w_out, a per-column constant: add on host
    bv_all = np.asarray(b_in).reshape(16, 3, PH)[:, 2, :].reshape(DIM)
    const = np.asarray(b_out) + bv_all @ np.asarray(w_out)
    out = np.empty((B, T, DIM), np.float32)
    for b in range(B):
        out[b] = results[2 * b]["y"] + results[2 * b + 1]["y"] + const[None, :]
    return out


# ---------------- self-contained kernel() entry point ----------------

_CACHED = {}


def _get_nc():
    if "nc" not in _CACHED:
        _CACHED["nc"] = build_kernel(T=2048, num_devices=8)
    return _CACHED["nc"]


def kernel(x, w_in, b_in, w_out, b_out):
    """Full-input MHA forward on 8 NeuronCores.

    x: (4, 2048, 768) f32; w_in: (768, 2304); b_in: (2304,);
    w_out: (768, 768); b_out: (768,). Returns (4, 2048, 768) f32.
    """
    from concourse.bass_utils import run_bass_kernel_spmd

    x = np.asarray(x, np.float32)
    w_in = np.asarray(w_in, np.float32)
    b_in = np.asarray(b_in, np.float32)
    w_out = np.asarray(w_out, np.float32)
    b_out = np.asarray(b_out, np.float32)

    nc = _get_nc()
    in_maps = host_prep(x, w_in, b_in, w_out, T=2048)
    res = run_bass_kernel_spmd(nc, in_maps, core_ids=list(range(8)))
    return host_post(res.results, b_out, b_in, w_out, B=4, T=2048)
